# revision 14
# baseline (speedup 1.0000x reference)
"""Trainium2 Bass kernel for the CRF forward algorithm (nn_CRF).

Reference: scan over S=8192 steps of
    fv'[i] = logsumexp_j(fv[j] + transitions[i, j]) + h[s, i]
then logsumexp(fv + transitions[END_IDX]).

Distribution (no cross-core communication): the step maps are products of
strictly positive matrices, so the normalized forward state forgets its
init at ~0.04x/step (measured: < 1e-13 deviation by step 12).  The 8192
steps split into 8 chunks stitched only by scalars: core c scans h rows
[c*1022, c*1022 + 1038); 16 burn-in steps from a zeros init (core 0: the
true CRF init), captures the state at local step 16 ("y" = global state
at the previous core's end cut) and at local step 1038 ("X" = its own
end cut; the last 2 steps are a straight epilogue with no normalization).
Each core's y has the same direction as the previous core's X, so the
unknown additive constants follow by matching LSEs on the host, which
also runs the terminal logsumexp in float64.

Device algorithm: exp-space state, lazy normalization (every 4 steps,
applied with a 1-iteration lag as an exp bias).  Critical path per step
is just PE mv-block -> DVE multiply -> PE mv-block.

Per-step math (state w = exp of the running log-state, bf16, scale drifts
and is renormalized every 4th step):
    E    = W @ w                      (PE, 256 MMs, fp32 psum)
    w'   = E * exp(h[s] + bias)       (ACT precomputes eh; DVE multiplies)
    bias = -ln(sigma E) from the previous iteration's skinny matmul,
           applied only at u=0; sigma E measured at u=3 (16 skinny MMs);
           C += ln(sigma E) bookkept on DVE; ACT computes the Ln.
Captures of (ln(w)+eps, C) at step BURN and at the end; host stitches the
8 chunk scans by scalar matching (see kernel.py docstring for the chunked
scheme; invariants I = ln(w) + C are exact at iteration boundaries).
"""
import sys

sys.path.insert(0, "/opt/trn_rl_repo")

import numpy as np

S = 8192
T = 2048
P = 128
NSLOT = T // P
NGRP = T // P
NBLK = NSLOT * NGRP
UNROLL = 4
EPS = 1e-30

NSTEPS = 1038           # 8*NSTEPS - 7*BURN == S
BURN = 16
STRIDE = NSTEPS - BURN  # 1022


def build_kernel(n_steps=NSTEPS, timing_mode=False, pe_only=False):
    import concourse.bacc as bacc
    import concourse.bass as bass
    import concourse.mybir as mybir
    from contextlib import ExitStack

    hsb_rows = 2 if timing_mode else n_steps
    assert n_steps % 2 == 0 and n_steps >= BURN + UNROLL
    epi = n_steps % UNROLL        # 2-step straight epilogue (no skinny/bias)
    n_iter = n_steps // UNROLL
    n_iter_a = BURN // UNROLL      # 4
    n_iter_b = n_iter - n_iter_a
    fp32 = mybir.dt.float32
    bf16 = mybir.dt.bfloat16
    AF = mybir.ActivationFunctionType
    ALU = mybir.AluOpType
    AX = mybir.AxisListType

    nc = bacc.Bacc("TRN2", target_bir_lowering=True, num_devices=8)

    n_wtb = 2 if timing_mode else NBLK
    wtb = nc.declare_dram_parameter("wtb", [n_wtb, P, P], fp32, isOutput=False)
    hsb = nc.declare_dram_parameter("hsb", [hsb_rows, T], fp32, isOutput=False)
    v0f = nc.declare_dram_parameter("v0f", [P, NSLOT], fp32, isOutput=False)
    out_y = nc.declare_dram_parameter("out_y", [P, NSLOT], fp32, isOutput=True)
    out_v = nc.declare_dram_parameter("out_v", [P, NSLOT], fp32, isOutput=True)
    out_s = nc.declare_dram_parameter("out_s", [1, 2], fp32, isOutput=True)

    ctx = ExitStack()
    sb = lambda name, shape, dt: ctx.enter_context(nc.sbuf_tensor(name, shape, dt))
    ps = lambda name, shape, dt: ctx.enter_context(nc.psum_tensor(name, shape, dt))
    sem = lambda name: ctx.enter_context(nc.semaphore(name))

    with ctx:
        wt = sb("wt", [P, NBLK * P], bf16)
        colsum = sb("colsum", [P, NSLOT], fp32)
        colsum_bf = sb("colsum_bf", [P, NSLOT], bf16)
        w2 = [sb("w_a", [P, NSLOT], bf16), sb("w_b", [P, NSLOT], bf16)]
        eh = [sb(f"eh{u}", [P, NSLOT], bf16) for u in range(UNROLL)]
        h_step = [sb(f"h_step{u}", [P, NSLOT], fp32) for u in range(UNROLL)]
        v0sb = sb("v0sb", [P, NSLOT], fp32)
        tmp = [sb(f"tmp{i}", [P, P], fp32) for i in range(2)]
        eps_t = sb("eps_t", [P, 1], fp32)
        neg_row = sb("neg_row", [1, P], fp32)
        nmh_sb = sb("nmh_sb", [P, 1], fp32)
        m_sb = sb("m_sb", [1, 1], fp32)
        c_acc = sb("c_acc", [1, 1], fp32)
        ysnap = sb("ysnap", [P, NSLOT], fp32)
        vend = sb("vend", [P, NSLOT], fp32)
        sc_out = sb("sc_out", [1, 2], fp32)

        psum_mv = ps("psum_mv", [P, NSLOT], fp32)
        psum_m = ps("psum_m", [1, 1], fp32)
        psum_b = ps("psum_b", [P, 1], fp32)

        su_dma = [sem("su_dma0"), sem("su_dma1")]
        su_exp = sem("su_exp")
        su_misc = sem("su_misc")
        v0_sem = sem("v0_sem")
        h_ready = [sem(f"h_ready{u}") for u in range(UNROLL)]
        act_eh = sem("act_eh")     # +1 per eh exp (per step)
        act_ln = sem("act_ln")     # +1 per iteration (m_sb valid)
        pe1 = sem("pe1")           # +1 per step (mv block done)
        pe_sig = sem("pe_sig")     # +1 per iteration (skinny done)
        pe2 = sem("pe2")           # +1 per iteration (bcast done)
        dve_st = sem("dve_st")     # +1 per step (w written, psum consumed)
        cap_sem = sem("cap_sem")
        fin_sem = sem("fin_sem")

        with nc.Block() as block:

            # ---------------- sync: input DMAs ----------------
            @block.sync
            def _(eng):
                eng.dma_start(v0sb[:, :], v0f[:, :]).then_inc(v0_sem, 16)
                for t in range(NBLK):
                    if t >= 2:
                        eng.wait_ge(su_exp, t - 1)
                    eng.dma_start(
                        tmp[t % 2][:, :],
                        wtb[(t % 2 if timing_mode else t), :, :],
                    ).then_inc(su_dma[t % 2], 16)
                for u in range(UNROLL):
                    eng.dma_start(
                        h_step[u][:, :],
                        hsb[(0 if timing_mode else u) : (1 if timing_mode else u + 1), :],
                    ).then_inc(h_ready[u], 16)
                r_off = eng.alloc_register("r_off")
                r_g = eng.alloc_register("r_g")
                r_i = eng.alloc_register("r_i")
                eng.reg_mov(r_off, 0 if timing_mode else UNROLL)
                eng.reg_mov(r_g, 0)
                eng.reg_mov(r_i, 0)
                eng.br("sync_done" if pe_only else "sync_loop")
                with nc.body("sync_loop"):
                    for u in range(UNROLL):
                        eng.reg_add(r_g, r_g, 1)
                        eng.wait_ge(act_eh, r_g)
                        eng.dma_start(
                            h_step[u][:, :],
                            hsb[bass.ds(eng.snap(r_off), 1), :],
                        ).then_inc(h_ready[u], 16)
                        if not timing_mode:
                            eng.reg_add(r_off, r_off, 1)
                    eng.reg_add(r_i, r_i, 1)
                    eng.br_lt(r_i, n_iter - 1, "sync_loop", "sync_epi")
                with nc.body("sync_epi"):
                    for e in range(epi):
                        eng.reg_add(r_g, r_g, 1)
                        eng.wait_ge(act_eh, r_g)
                        eng.dma_start(
                            h_step[e][:, :],
                            hsb[bass.ds(eng.snap(r_off), 1), :],
                        ).then_inc(h_ready[e], 16)
                        if not timing_mode:
                            eng.reg_add(r_off, r_off, 1)
                    eng.br("sync_done")
                with nc.body("sync_done"):
                    eng.wait_ge(fin_sem, 2)
                    eng.dma_start(out_y[:, :], ysnap[:, :]).then_inc(su_misc, 16)
                    eng.dma_start(out_v[:, :], vend[:, :]).then_inc(su_misc, 16)
                    eng.dma_start(out_s[:, :], sc_out[:, :]).then_inc(su_misc, 16)
                    eng.br(block.end_bb)

            # ---------------- gpsimd: constants ----------------
            @block.gpsimd
            def _(eng):
                eng.memset(eps_t[:, :], EPS)
                eng.memset(neg_row[:, :], -1.0)
                eng.memset(c_acc[:, :], 0.0)
                eng.memset(m_sb[:, :], 0.0)
                eng.drain()
                eng.nop().then_inc(su_misc, 16)

            # ---------------- scalar (ACT) ----------------
            @block.scalar
            def _(eng):
                for t in range(NBLK):
                    eng.wait_ge(su_dma[t % 2], 16 * (t // 2 + 1))
                    eng.activation(
                        wt[:, t * P : (t + 1) * P], tmp[t % 2][:, :], AF.Exp
                    ).then_inc(su_exp, 1)
                eng.wait_ge(v0_sem, 16)
                eng.wait_ge(su_misc, 32)
                eng.activation(w2[0][:, :], v0sb[:, :], AF.Exp).then_inc(
                    su_misc, 16
                )
                if pe_only:
                    eng.br(block.end_bb)
                    return
                r_h = eng.alloc_register("r_h")      # h_ready target
                r_p2 = eng.alloc_register("r_p2")    # pe2 target
                r_war = eng.alloc_register("r_war")  # dve_st (eh WAR)
                r_sig = eng.alloc_register("r_sig")  # pe_sig target
                r_dl = eng.alloc_register("r_dl")    # dve_st (m_sb WAR)
                r_it = eng.alloc_register("r_it")
                # peeled iteration 0
                eng.wait_ge(h_ready[0], 16)
                eng.wait_ge(pe2, 1)
                eng.activation(nmh_sb[:, :], psum_b[:, :], AF.Copy)
                eng.drain()
                eng.activation(
                    eh[0][:, :], h_step[0][:, :], AF.Exp, bias=nmh_sb[:, :]
                ).then_inc(act_eh, 1)
                for u in range(1, UNROLL):
                    eng.wait_ge(h_ready[u], 16)
                    eng.activation(
                        eh[u][:, :], h_step[u][:, :], AF.Exp
                    ).then_inc(act_eh, 1)
                eng.wait_ge(pe_sig, 1)
                eng.wait_ge(dve_st, 1)
                eng.activation(m_sb[:, :], psum_m[:, :], AF.Ln).then_inc(
                    act_ln, 1
                )
                eng.reg_mov(r_h, 16)
                eng.reg_mov(r_p2, 1)
                eng.reg_mov(r_war, 0)
                eng.reg_mov(r_sig, 1)
                eng.reg_mov(r_dl, 1)
                eng.reg_mov(r_it, 0)

                def act_iter():
                    eng.reg_add(r_h, r_h, 16)
                    eng.reg_add(r_p2, r_p2, 1)
                    for u in range(UNROLL):
                        eng.reg_add(r_war, r_war, 1)
                        eng.wait_ge(dve_st, r_war)
                        eng.wait_ge(h_ready[u], r_h)
                        if u == 0:
                            eng.wait_ge(pe2, r_p2)
                            eng.activation(nmh_sb[:, :], psum_b[:, :], AF.Copy)
                            eng.drain()
                            eng.activation(
                                eh[0][:, :], h_step[0][:, :], AF.Exp,
                                bias=nmh_sb[:, :],
                            ).then_inc(act_eh, 1)
                        else:
                            eng.activation(
                                eh[u][:, :], h_step[u][:, :], AF.Exp
                            ).then_inc(act_eh, 1)
                    eng.reg_add(r_sig, r_sig, 1)
                    eng.reg_add(r_dl, r_dl, 4)
                    eng.wait_ge(pe_sig, r_sig)
                    eng.wait_ge(dve_st, r_dl)
                    eng.activation(m_sb[:, :], psum_m[:, :], AF.Ln).then_inc(
                        act_ln, 1
                    )

                eng.br("act_loop_a")
                with nc.body("act_loop_a"):
                    act_iter()
                    eng.reg_add(r_it, r_it, 1)
                    eng.br_lt(r_it, n_iter_a - 1, "act_loop_a", "act_cap")
                with nc.body("act_cap"):
                    eng.wait_ge(dve_st, BURN)
                    eng.activation(
                        ysnap[:, :], w2[0][:, :], AF.Ln, bias=eps_t[:, :]
                    ).then_inc(cap_sem, 1)
                    eng.reg_mov(r_it, 0)
                    eng.br("act_loop_b")
                with nc.body("act_loop_b"):
                    act_iter()
                    eng.reg_add(r_it, r_it, 1)
                    eng.br_lt(r_it, n_iter_b, "act_loop_b", "act_fin")
                with nc.body("act_fin"):
                    if epi:
                        eng.reg_add(r_h, r_h, 16)
                        for e in range(epi):
                            eng.reg_add(r_war, r_war, 1)
                            eng.wait_ge(dve_st, r_war)
                            eng.wait_ge(h_ready[e], r_h)
                            eng.activation(
                                eh[e][:, :], h_step[e][:, :], AF.Exp
                            ).then_inc(act_eh, 1)
                    eng.wait_ge(dve_st, n_steps)
                    eng.activation(
                        vend[:, :], w2[0][:, :], AF.Ln, bias=eps_t[:, :]
                    ).then_inc(fin_sem, 1)
                    eng.br(block.end_bb)

            # ---------------- tensor (PE) ----------------
            @block.tensor
            def _(eng):
                eng.wait_ge(su_misc, 48)

                def mv_block(u):
                    wbuf = w2[u % 2]
                    for g in range(NGRP):
                        for k in range(NSLOT):
                            t = k * NGRP + g
                            mm = eng.matmul(
                                psum_mv[:, g : g + 1],
                                wt[:, t * P : (t + 1) * P],
                                wbuf[:, k : k + 1],
                                start=(k == 0),
                                stop=(k == NSLOT - 1),
                            )
                            if g == NGRP - 1 and k == NSLOT - 1:
                                mm.then_inc(pe1, 1)

                def skinny(u):
                    wbuf = w2[u % 2]
                    for k in range(NSLOT):
                        mm = eng.matmul(
                            psum_m[:, :],
                            colsum_bf[:, k : k + 1],
                            wbuf[:, k : k + 1],
                            start=(k == 0),
                            stop=(k == NSLOT - 1),
                        )
                        if k == NSLOT - 1:
                            mm.then_inc(pe_sig, 1)

                # peeled iteration 0
                eng.matmul(
                    psum_b[:, :], neg_row[:, :], m_sb[:, :], start=True,
                    stop=True,
                ).then_inc(pe2, 1)
                for u in range(UNROLL):
                    if u > 0 and not pe_only:
                        eng.wait_ge(dve_st, u)
                    mv_block(u)
                    if u == UNROLL - 1:
                        skinny(u)
                r_ln = eng.alloc_register("r_ln")
                r_aeh = eng.alloc_register("r_aeh")
                r_dve = eng.alloc_register("r_dve")
                r_it = eng.alloc_register("r_it")
                eng.reg_mov(r_ln, 1)
                eng.reg_mov(r_aeh, 1)
                eng.reg_mov(r_dve, 3)
                eng.reg_mov(r_it, 0)
                eng.br("pe_loop")
                with nc.body("pe_loop"):
                    if not pe_only:
                        eng.wait_ge(act_ln, r_ln)
                        eng.wait_ge(act_eh, r_aeh)
                    eng.reg_add(r_ln, r_ln, 1)
                    eng.reg_add(r_aeh, r_aeh, 4)
                    eng.matmul(
                        psum_b[:, :], neg_row[:, :], m_sb[:, :], start=True,
                        stop=True,
                    ).then_inc(pe2, 1)
                    for u in range(UNROLL):
                        eng.reg_add(r_dve, r_dve, 1)
                        if not pe_only:
                            eng.wait_ge(dve_st, r_dve)
                        mv_block(u)
                        if u == UNROLL - 1:
                            skinny(u)
                    eng.reg_add(r_it, r_it, 1)
                    eng.br_lt(r_it, n_iter - 1, "pe_loop", "pe_fin")
                with nc.body("pe_fin"):
                    for e in range(epi):
                        eng.reg_add(r_dve, r_dve, 1)
                        if not pe_only:
                            eng.wait_ge(dve_st, r_dve)
                        mv_block(e)
                    eng.br(block.end_bb)

            # ---------------- vector (DVE) ----------------
            @block.vector
            def _(eng):
                for k in range(NSLOT):
                    eng.wait_ge(su_exp, (k + 1) * NGRP)
                    eng.tensor_reduce(
                        colsum[:, k : k + 1],
                        wt[:, k * NGRP * P : (k + 1) * NGRP * P],
                        axis=AX.X,
                        op=ALU.add,
                    )
                eng.drain()
                eng.tensor_copy(colsum_bf[:, :], colsum[:, :]).then_inc(
                    su_misc, 16
                )
                eng.wait_ge(su_misc, 32)
                if pe_only:
                    eng.nop().then_inc(fin_sem, 2)
                    eng.br(block.end_bb)
                    return
                r_p1 = eng.alloc_register("r_p1")
                r_aeh = eng.alloc_register("r_aeh")
                r_ln = eng.alloc_register("r_ln")
                r_it = eng.alloc_register("r_it")
                eng.reg_mov(r_p1, 0)
                eng.reg_mov(r_aeh, 0)
                eng.reg_mov(r_ln, 0)
                eng.reg_mov(r_it, 0)

                def dve_iter():
                    for u in range(UNROLL):
                        eng.reg_add(r_p1, r_p1, 1)
                        eng.reg_add(r_aeh, r_aeh, 1)
                        eng.wait_ge(pe1, r_p1)
                        eng.wait_ge(act_eh, r_aeh)
                        if u == 0:
                            eng.wait_ge(act_ln, r_ln)
                            eng.reg_add(r_ln, r_ln, 1)
                            eng.tensor_tensor(
                                c_acc[:, :], c_acc[:, :], m_sb[:, :],
                                op=ALU.add,
                            )
                        eng.tensor_tensor(
                            w2[(u + 1) % 2][:, :], psum_mv[:, :], eh[u][:, :],
                            op=ALU.mult,
                        ).then_inc(dve_st, 1)

                eng.br("dve_loop_a")
                with nc.body("dve_loop_a"):
                    dve_iter()
                    eng.reg_add(r_it, r_it, 1)
                    eng.br_lt(r_it, n_iter_a, "dve_loop_a", "dve_cap")
                with nc.body("dve_cap"):
                    eng.drain()
                    eng.tensor_copy(sc_out[:, 0:1], c_acc[:, :])
                    eng.wait_ge(cap_sem, 1)
                    eng.reg_mov(r_it, 0)
                    eng.br("dve_loop_b")
                with nc.body("dve_loop_b"):
                    dve_iter()
                    eng.reg_add(r_it, r_it, 1)
                    eng.br_lt(r_it, n_iter_b, "dve_loop_b", "dve_fin")
                with nc.body("dve_fin"):
                    for e in range(epi):
                        eng.reg_add(r_p1, r_p1, 1)
                        eng.reg_add(r_aeh, r_aeh, 1)
                        eng.wait_ge(pe1, r_p1)
                        eng.wait_ge(act_eh, r_aeh)
                        eng.tensor_tensor(
                            w2[(e + 1) % 2][:, :], psum_mv[:, :], eh[e][:, :],
                            op=ALU.mult,
                        ).then_inc(dve_st, 1)
                    eng.drain()
                    eng.tensor_copy(sc_out[:, 1:2], c_acc[:, :])
                    eng.drain()
                    eng.nop().then_inc(fin_sem, 1)
                    eng.br(block.end_bb)

    nc.compile()
    return nc


_NC_CACHE = {}


def _get_nc(n_steps=NSTEPS, timing_mode=False, pe_only=False):
    key = (n_steps, timing_mode, pe_only)
    if key not in _NC_CACHE:
        _NC_CACHE[key] = build_kernel(
            n_steps, timing_mode=timing_mode, pe_only=pe_only
        )
    return _NC_CACHE[key]


def prep_inputs(h, transitions):
    h = np.ascontiguousarray(np.asarray(h, dtype=np.float32))
    tr = np.ascontiguousarray(np.asarray(transitions, dtype=np.float32))
    assert h.shape == (S, T) and tr.shape == (T, T)
    wtb = np.empty((NBLK, P, P), dtype=np.float32)
    for k in range(NSLOT):
        for g in range(NGRP):
            wtb[k * NGRP + g] = tr[g::NGRP, :][:, k::NSLOT].T
    wtb = np.ascontiguousarray(wtb)
    v0_true = np.full((T,), -10000.0, dtype=np.float32)
    v0_true[0] = 0.0
    in_maps = []
    for c in range(8):
        o = c * STRIDE
        v0 = v0_true if c == 0 else np.zeros((T,), dtype=np.float32)
        in_maps.append(
            {
                "wtb": wtb,
                "hsb": np.ascontiguousarray(h[o : o + NSTEPS]),
                "v0f": np.ascontiguousarray(v0.reshape(P, NSLOT)),
            }
        )
    return in_maps


def _lse64(x):
    m = x.max()
    return m + np.log(np.exp(x - m).sum())


def stitch(results, transitions):
    tr_end = np.asarray(transitions, dtype=np.float64)[1]
    kappa = 0.0
    prev = None
    for c in range(8):
        r = results[c]
        v_y = np.asarray(r["out_y"], dtype=np.float64).reshape(T)
        v_end = np.asarray(r["out_v"], dtype=np.float64).reshape(T)
        c_y = float(r["out_s"][0, 0])
        c_end = float(r["out_s"][0, 1])
        if c > 0:
            pv, pc = prev
            kappa += (pc + _lse64(pv)) - (c_y + _lse64(v_y))
        prev = (v_end, c_end)
    v8, c8 = prev
    return np.float32(_lse64(v8 + tr_end) + c8 + kappa)


def kernel(h, transitions):
    from concourse.bass_utils import run_bass_kernel_spmd

    in_maps = prep_inputs(h, transitions)
    nc = _get_nc()
    res = run_bass_kernel_spmd(nc, in_maps, list(range(8)))
    return stitch(res.results, transitions)


if __name__ == "__main__":
    from ref_numpy import get_inputs

    inputs = get_inputs()
    out = kernel(**inputs)
    print("kernel out:", out)
